# revision 7
# baseline (speedup 1.0000x reference)
"""DistMult scoring kernel for Trainium2 (8 NeuronCores, data-parallel).

score[b] = sum_d src[b,d] * rel[d] * dst[b,d],  rel = M[rel_idx]

Sharding: batch dim split evenly across 8 cores; rel row replicated.
Per-core dataflow (rows on partitions, D on free dim):
  - DMA [128, G, 128] fp32 tiles of src/dst (G row-blocks per ~1MB DMA)
  - DVE:   prod = src * dst          (one tensor_tensor over the group)
  - DVE:   prod = prod * rel_bcast   (rel replicated across partitions,
                                      stride-0 broadcast along the group dim)
  - ACT:   per 128-row block, activation(Copy) with accum_out -> row sums
  - scores collect in an SBUF [128, n_blocks] tile; PE-transposes them into
    PSUM so the final DMAs write contiguous 512B rows per partition.
"""

import numpy as np

N_CORES = 8
B = 500000
D = 128
P = 128
ROWS = B // N_CORES  # 62500 rows per core
GROUP = 16  # 128-row blocks per DMA group (~1MB per tensor per group)

_CACHE = {}


def _build(rows, group):
    import concourse.bacc as bacc
    import concourse.tile as tile
    from concourse import masks, mybir

    f32 = mybir.dt.float32

    nc = bacc.Bacc(
        "TRN2", target_bir_lowering=False, debug=False, num_devices=N_CORES
    )
    src = nc.dram_tensor("src", [rows, D], f32, kind="ExternalInput").ap()
    dst = nc.dram_tensor("dst", [rows, D], f32, kind="ExternalInput").ap()
    rel = nc.dram_tensor("rel", [1, D], f32, kind="ExternalInput").ap()
    out = nc.dram_tensor("out", [rows], f32, kind="ExternalOutput").ap()

    n_blocks = rows // P
    tail = rows - n_blocks * P

    with tile.TileContext(nc) as tc:
        with (
            tc.tile_pool(name="io", bufs=4) as io_pool,
            tc.tile_pool(name="prod", bufs=3) as prod_pool,
            tc.tile_pool(name="consts", bufs=1) as consts,
            tc.tile_pool(name="psum", bufs=2, space="PSUM") as psum_pool,
        ):
            relrep = consts.tile([P, D], f32)
            nc.sync.dma_start(out=relrep[:], in_=rel[0:1, :].broadcast_to([P, D]))
            dummy = consts.tile([P, D], f32)
            ident = consts.tile([P, P], f32)
            masks.make_identity(nc, ident[:])
            score_cols = n_blocks + (1 if tail else 0)
            scores = consts.tile([P, score_cols], f32)

            col = 0
            b0 = 0
            while b0 < n_blocks:
                g = min(group, n_blocks - b0)
                s = io_pool.tile([P, g, D], f32, tag="s")
                d_ = io_pool.tile([P, g, D], f32, tag="d")
                nc.sync.dma_start(
                    out=s[:],
                    in_=src[b0 * P : (b0 + g) * P, :].rearrange(
                        "(j p) d -> p j d", p=P
                    ),
                )
                nc.sync.dma_start(
                    out=d_[:],
                    in_=dst[b0 * P : (b0 + g) * P, :].rearrange(
                        "(j p) d -> p j d", p=P
                    ),
                )
                pr = prod_pool.tile([P, g, D], f32, tag="pr")
                relb = relrep[:].unsqueeze(1).broadcast_to([P, g, D])
                nc.vector.tensor_tensor(pr[:], s[:], relb, mybir.AluOpType.mult)
                nc.vector.tensor_tensor(pr[:], pr[:], d_[:], mybir.AluOpType.mult)
                for j in range(g):
                    nc.scalar.activation(
                        dummy[:],
                        pr[:, j, :],
                        mybir.ActivationFunctionType.Copy,
                        accum_out=scores[:, col : col + 1],
                    )
                    col += 1
                b0 += g

            if tail:
                st = io_pool.tile([tail, D], f32, tag="s_tail")
                dt_ = io_pool.tile([tail, D], f32, tag="d_tail")
                nc.sync.dma_start(out=st[:], in_=src[n_blocks * P : rows, :])
                nc.sync.dma_start(out=dt_[:], in_=dst[n_blocks * P : rows, :])
                prt = prod_pool.tile([tail, D], f32, tag="pr_tail")
                nc.vector.tensor_tensor(
                    prt[:], st[:], relrep[0:tail, :], mybir.AluOpType.mult
                )
                nc.vector.tensor_tensor(prt[:], prt[:], dt_[:], mybir.AluOpType.mult)
                nc.scalar.activation(
                    dummy[0:tail, :],
                    prt[:],
                    mybir.ActivationFunctionType.Copy,
                    accum_out=scores[0:tail, n_blocks : n_blocks + 1],
                )

            # Transpose score columns so output DMAs are contiguous per row.
            for c0 in range(0, n_blocks, P):
                cc = min(P, n_blocks - c0)
                pt = psum_pool.tile([P, P], f32, tag="pt")
                nc.tensor.transpose(pt[0:cc, :], scores[:, c0 : c0 + cc], ident[:])
                sb = prod_pool.tile([P, P], f32, tag="scoresT")
                nc.vector.tensor_copy(sb[0:cc, :], pt[0:cc, :])
                nc.scalar.dma_start(
                    out=out[c0 * P : (c0 + cc) * P].rearrange("(t p) -> t p", p=P),
                    in_=sb[0:cc, :],
                )
            if tail:
                nc.scalar.dma_start(
                    out=out[n_blocks * P : rows].rearrange("(p x) -> p x", x=1),
                    in_=scores[0:tail, n_blocks : n_blocks + 1],
                )
    nc.compile()
    return nc


def _get_program(rows, group):
    key = (rows, group)
    if key not in _CACHE:
        _CACHE[key] = _build(rows, group)
    return _CACHE[key]


def kernel(src_emb, dst_emb, M, rel_idx):
    from concourse.bass_utils import run_bass_kernel_spmd

    src_emb = np.asarray(src_emb, dtype=np.float32)
    dst_emb = np.asarray(dst_emb, dtype=np.float32)
    M = np.asarray(M, dtype=np.float32)
    rel = np.ascontiguousarray(M[int(rel_idx)]).reshape(1, D)

    nc = _get_program(ROWS, GROUP)
    in_maps = [
        {
            "src": np.ascontiguousarray(src_emb[i * ROWS : (i + 1) * ROWS]),
            "dst": np.ascontiguousarray(dst_emb[i * ROWS : (i + 1) * ROWS]),
            "rel": rel,
        }
        for i in range(N_CORES)
    ]
    res = run_bass_kernel_spmd(nc, in_maps, list(range(N_CORES)))
    out = np.concatenate([res.results[i]["out"] for i in range(N_CORES)])
    return out.reshape(B, 1, 1).astype(np.float32)


# revision 10
# speedup vs baseline: 30.7386x; 30.7386x over previous
"""DistMult scoring kernel for Trainium2 (8 NeuronCores, data-parallel).

score[b] = sum_d src[b,d] * rel[d] * dst[b,d],  rel = M[rel_idx]

Sharding: batch dim split evenly across 8 cores; rel row replicated.
Per-core dataflow (rows on partitions, D on free dim):
  - DMA [128, G, 128] fp32 tiles of src/dst (G row-blocks per ~1MB DMA)
  - DVE:   prod = src * dst          (one tensor_tensor over the group)
  - DVE:   prod = prod * rel_bcast   (rel replicated across partitions,
                                      stride-0 broadcast along the group dim)
  - ACT:   per 128-row block, activation(Copy) with accum_out -> row sums
  - scores collect in an SBUF [128, n_blocks] tile; PE-transposes them into
    PSUM so the final DMAs write contiguous 512B rows per partition.
"""

import numpy as np

N_CORES = 8
B = 500000
D = 128
P = 128
ROWS = B // N_CORES  # 62500 rows per core
GROUP = 16  # 128-row blocks per DMA group (~1MB per tensor per group)

_CACHE = {}


def _build(rows, group, repeat=1):
    """Build the per-core program. repeat>1 wraps the whole body in a HW
    loop — used only for wall-clock benchmarking (amortizes dispatch
    overhead); the graded kernel uses repeat=1."""
    import contextlib

    import concourse.bacc as bacc
    import concourse.tile as tile
    from concourse import masks, mybir

    f32 = mybir.dt.float32

    nc = bacc.Bacc(
        "TRN2", target_bir_lowering=False, debug=False, num_devices=N_CORES
    )
    src = nc.dram_tensor("src", [rows, D], f32, kind="ExternalInput").ap()
    dst = nc.dram_tensor("dst", [rows, D], f32, kind="ExternalInput").ap()
    rel = nc.dram_tensor("rel", [1, D], f32, kind="ExternalInput").ap()
    out = nc.dram_tensor("out", [rows], f32, kind="ExternalOutput").ap()

    n_blocks = rows // P
    tail = rows - n_blocks * P

    with tile.TileContext(nc) as tc:
        with (
            tc.tile_pool(name="io", bufs=4) as io_pool,
            tc.tile_pool(name="prod", bufs=3) as prod_pool,
            tc.tile_pool(name="consts", bufs=1) as consts,
            tc.tile_pool(name="psum", bufs=2, space="PSUM") as psum_pool,
        ):
            relrep = consts.tile([P, D], f32)
            nc.sync.dma_start(out=relrep[:], in_=rel[0:1, :].broadcast_to([P, D]))
            dummy = consts.tile([P, D], f32)
            ident = consts.tile([P, P], f32)
            masks.make_identity(nc, ident[:])
            score_cols = n_blocks + (1 if tail else 0)
            scores = consts.tile([P, score_cols], f32)

            loop = (
                tc.For_i(0, repeat, 1)
                if repeat > 1
                else contextlib.nullcontext()
            )
            with loop:
                _emit_body(
                    nc, tc, io_pool, prod_pool, psum_pool,
                    src, dst, out, relrep, dummy, ident, scores,
                    rows, group, n_blocks, tail, f32,
                )
    nc.compile()
    return nc


def _emit_body(
    nc, tc, io_pool, prod_pool, psum_pool,
    src, dst, out, relrep, dummy, ident, scores,
    rows, group, n_blocks, tail, f32,
):
    from concourse import mybir

    if True:
        if True:
            col = 0
            b0 = 0
            while b0 < n_blocks:
                g = min(group, n_blocks - b0)
                s = io_pool.tile([P, g, D], f32, tag="s")
                d_ = io_pool.tile([P, g, D], f32, tag="d")
                nc.sync.dma_start(
                    out=s[:],
                    in_=src[b0 * P : (b0 + g) * P, :].rearrange(
                        "(j p) d -> p j d", p=P
                    ),
                )
                nc.sync.dma_start(
                    out=d_[:],
                    in_=dst[b0 * P : (b0 + g) * P, :].rearrange(
                        "(j p) d -> p j d", p=P
                    ),
                )
                pr = prod_pool.tile([P, g, D], f32, tag="pr")
                relb = relrep[:].unsqueeze(1).broadcast_to([P, g, D])
                nc.vector.tensor_tensor(pr[:], s[:], relb, mybir.AluOpType.mult)
                nc.vector.tensor_tensor(pr[:], pr[:], d_[:], mybir.AluOpType.mult)
                for j in range(g):
                    nc.scalar.activation(
                        dummy[:],
                        pr[:, j, :],
                        mybir.ActivationFunctionType.Copy,
                        accum_out=scores[:, col : col + 1],
                    )
                    col += 1
                b0 += g

            if tail:
                st = io_pool.tile([tail, D], f32, tag="s_tail")
                dt_ = io_pool.tile([tail, D], f32, tag="d_tail")
                nc.sync.dma_start(out=st[:], in_=src[n_blocks * P : rows, :])
                nc.sync.dma_start(out=dt_[:], in_=dst[n_blocks * P : rows, :])
                prt = prod_pool.tile([tail, D], f32, tag="pr_tail")
                nc.vector.tensor_tensor(
                    prt[:], st[:], relrep[0:tail, :], mybir.AluOpType.mult
                )
                nc.vector.tensor_tensor(prt[:], prt[:], dt_[:], mybir.AluOpType.mult)
                nc.scalar.activation(
                    dummy[0:tail, :],
                    prt[:],
                    mybir.ActivationFunctionType.Copy,
                    accum_out=scores[0:tail, n_blocks : n_blocks + 1],
                )

            # Transpose score columns so output DMAs are contiguous per row.
            for c0 in range(0, n_blocks, P):
                cc = min(P, n_blocks - c0)
                pt = psum_pool.tile([P, P], f32, tag="pt")
                nc.tensor.transpose(pt[0:cc, :], scores[:, c0 : c0 + cc], ident[:])
                sb = prod_pool.tile([P, P], f32, tag="scoresT")
                nc.vector.tensor_copy(sb[0:cc, :], pt[0:cc, :])
                nc.scalar.dma_start(
                    out=out[c0 * P : (c0 + cc) * P].rearrange("(t p) -> t p", p=P),
                    in_=sb[0:cc, :],
                )
            if tail:
                nc.scalar.dma_start(
                    out=out[n_blocks * P : rows].rearrange("(p x) -> p x", x=1),
                    in_=scores[0:tail, n_blocks : n_blocks + 1],
                )


def _get_program(rows, group):
    key = (rows, group)
    if key not in _CACHE:
        _CACHE[key] = _build(rows, group)
    return _CACHE[key]


def kernel(src_emb, dst_emb, M, rel_idx):
    from concourse.bass_utils import run_bass_kernel_spmd

    src_emb = np.asarray(src_emb, dtype=np.float32)
    dst_emb = np.asarray(dst_emb, dtype=np.float32)
    M = np.asarray(M, dtype=np.float32)
    rel = np.ascontiguousarray(M[int(rel_idx)]).reshape(1, D)

    nc = _get_program(ROWS, GROUP)
    in_maps = [
        {
            "src": np.ascontiguousarray(src_emb[i * ROWS : (i + 1) * ROWS]),
            "dst": np.ascontiguousarray(dst_emb[i * ROWS : (i + 1) * ROWS]),
            "rel": rel,
        }
        for i in range(N_CORES)
    ]
    res = run_bass_kernel_spmd(nc, in_maps, list(range(N_CORES)))
    out = np.concatenate([res.results[i]["out"] for i in range(N_CORES)])
    return out.reshape(B, 1, 1).astype(np.float32)


# revision 16
# speedup vs baseline: 50.7840x; 1.6521x over previous
"""DistMult scoring kernel for Trainium2 (8 NeuronCores, data-parallel).

score[b] = sum_d src[b,d] * rel[d] * dst[b,d],  rel = M[rel_idx]

Sharding: batch dim split evenly across 8 cores; rel row replicated.
Per-core dataflow ("colmajor" layout: partition p owns the contiguous row
span [p*C, (p+1)*C), C = rows//128, so every DMA descriptor run is
g*512B contiguous per partition and the output is ONE contiguous DMA):
  - DMA [128, g=16, 128] fp32 tiles of src/dst (~1MB per dma_start, all
    on the otherwise-idle sync-engine HWDGE ring)
  - VectorE pass 1 (in place): s = s * rel_bcast  (rel replicated across
    partitions once; stride-0 broadcast along the group dim)
  - VectorE pass 2 (in place): s = s * dst
  - row-sum reduce, alternating per group between VectorE tensor_reduce
    and ScalarE activation-accumulate so neither engine exceeds the DMA
    streaming time (measured: ACT accum is 650ns/block on HW, 2.5x its
    modeled cost, so ACT alone would bottleneck; DVE alone just about
    fits; the split leaves slack on both)
  - scores[p, c] = score of row p*C + c -> single contiguous output DMA.
Measured ~195us/core vs the 179us HBM roofline (64MB in @ ~358GB/s/core).
"""

import numpy as np

N_CORES = 8
B = 500000
D = 128
P = 128
ROWS = B // N_CORES  # 62500 rows per core
GROUP = 16  # 128-row blocks per DMA group (~1MB per tensor per group)

# Kernel-structure knobs (resolved by experiments; see explore.py):
#   tt1/tt2: engine for the two elementwise passes ("vector" | "gpsimd")
#   reduce: "act" (ScalarE activation-accum per block), "dve" (one
#           tensor_reduce per group), or "split" (alternate groups)
#   rings:  1 = all input DMAs on the sync HWDGE ring; 2 = dst on scalar ring
#   inplace: write pass-1 output into the src tile (no separate prod pool)
#   layout: "rowmajor" (partition = row % 128; needs PE-transpose of scores)
#           "colmajor" (partition p owns rows [p*C, (p+1)*C); contiguous
#           8KB-per-partition DMA descriptors and a single contiguous
#           output DMA, no transpose)
DEFAULT_CFG = dict(
    tt1="vector", tt2="vector", reduce="split", rings=1, inplace=True,
    bufs_io=4, bufs_pr=3, layout="colmajor",
)

_CACHE = {}


def _build(rows, group, repeat=1, cfg=None):
    """Build the per-core program. repeat>1 wraps the whole body in a HW
    loop — used only for wall-clock benchmarking (amortizes dispatch
    overhead); the graded kernel uses repeat=1."""
    import contextlib

    import concourse.bacc as bacc
    import concourse.tile as tile
    from concourse import masks, mybir

    cfg = {**DEFAULT_CFG, **(cfg or {})}
    f32 = mybir.dt.float32

    nc = bacc.Bacc(
        "TRN2", target_bir_lowering=False, debug=False, num_devices=N_CORES
    )
    src = nc.dram_tensor("src", [rows, D], f32, kind="ExternalInput").ap()
    dst = nc.dram_tensor("dst", [rows, D], f32, kind="ExternalInput").ap()
    rel = nc.dram_tensor("rel", [1, D], f32, kind="ExternalInput").ap()
    out = nc.dram_tensor("out", [rows], f32, kind="ExternalOutput").ap()

    n_blocks = rows // P
    tail = rows - n_blocks * P

    with tile.TileContext(nc) as tc:
        with (
            tc.tile_pool(name="io", bufs=cfg["bufs_io"]) as io_pool,
            tc.tile_pool(name="prod", bufs=cfg["bufs_pr"]) as prod_pool,
            tc.tile_pool(name="consts", bufs=1) as consts,
            tc.tile_pool(name="psum", bufs=2, space="PSUM") as psum_pool,
        ):
            relrep = consts.tile([P, D], f32)
            nc.sync.dma_start(out=relrep[:], in_=rel[0:1, :].broadcast_to([P, D]))
            dummy = consts.tile([P, D], f32)
            ident = consts.tile([P, P], f32)
            masks.make_identity(nc, ident[:])
            score_cols = n_blocks + (1 if tail else 0)
            scores = consts.tile([P, score_cols], f32)

            loop = (
                tc.For_i(0, repeat, 1)
                if repeat > 1
                else contextlib.nullcontext()
            )
            with loop:
                _emit_body(
                    nc, tc, io_pool, prod_pool, psum_pool,
                    src, dst, out, relrep, dummy, ident, scores,
                    rows, group, n_blocks, tail, f32, cfg,
                )
    nc.compile()
    return nc


def _emit_body(
    nc, tc, io_pool, prod_pool, psum_pool,
    src, dst, out, relrep, dummy, ident, scores,
    rows, group, n_blocks, tail, f32, cfg,
):
    from concourse import mybir

    mult = mybir.AluOpType.mult
    eng = {"vector": nc.vector, "gpsimd": nc.gpsimd}
    tt1 = eng[cfg["tt1"]]
    tt2 = eng[cfg["tt2"]]
    dma_d = nc.scalar if cfg["rings"] == 2 else nc.sync
    colmajor = cfg["layout"] == "colmajor"
    if colmajor:
        # [P*n_blocks, D] viewed so partition p owns rows [p*n_blocks, ...)
        src_v = src[0 : P * n_blocks, :].rearrange("(p c) d -> p c d", p=P)
        dst_v = dst[0 : P * n_blocks, :].rearrange("(p c) d -> p c d", p=P)

    col = 0
    b0 = 0
    gi = 0
    while b0 < n_blocks:
        g = min(group, n_blocks - b0)
        s = io_pool.tile([P, g, D], f32, tag="s")
        d_ = io_pool.tile([P, g, D], f32, tag="d")
        if colmajor:
            in_s = src_v[:, b0 : b0 + g, :]
            in_d = dst_v[:, b0 : b0 + g, :]
        else:
            in_s = src[b0 * P : (b0 + g) * P, :].rearrange(
                "(j p) d -> p j d", p=P
            )
            in_d = dst[b0 * P : (b0 + g) * P, :].rearrange(
                "(j p) d -> p j d", p=P
            )
        nc.sync.dma_start(out=s[:], in_=in_s)
        dma_d.dma_start(out=d_[:], in_=in_d)
        if cfg["inplace"]:
            pr = s
        else:
            pr = prod_pool.tile([P, g, D], f32, tag="pr")
        relb = relrep[:].unsqueeze(1).broadcast_to([P, g, D])
        tt1.tensor_tensor(pr[:], s[:], relb, mult)
        tt2.tensor_tensor(pr[:], pr[:], d_[:], mult)
        mode = cfg["reduce"]
        if mode == "split":
            mode = "act" if gi % 2 == 0 else "dve"
        if mode == "act":
            for j in range(g):
                nc.scalar.activation(
                    dummy[:],
                    pr[:, j, :],
                    mybir.ActivationFunctionType.Copy,
                    accum_out=scores[:, col : col + 1],
                )
                col += 1
        else:  # dve
            nc.vector.tensor_reduce(
                scores[:, col : col + g],
                pr[:],
                axis=mybir.AxisListType.X,
                op=mybir.AluOpType.add,
            )
            col += g
        b0 += g
        gi += 1

    if tail:
        st = io_pool.tile([tail, D], f32, tag="s_tail")
        dt_ = io_pool.tile([tail, D], f32, tag="d_tail")
        nc.sync.dma_start(out=st[:], in_=src[n_blocks * P : rows, :])
        dma_d.dma_start(out=dt_[:], in_=dst[n_blocks * P : rows, :])
        prt = prod_pool.tile([tail, D], f32, tag="pr_tail")
        nc.vector.tensor_tensor(prt[:], st[:], relrep[0:tail, :], mult)
        nc.vector.tensor_tensor(prt[:], prt[:], dt_[:], mult)
        nc.scalar.activation(
            dummy[0:tail, :],
            prt[:],
            mybir.ActivationFunctionType.Copy,
            accum_out=scores[0:tail, n_blocks : n_blocks + 1],
        )

    if colmajor:
        # scores[p, c] is already the score of row p*n_blocks + c:
        # one contiguous output DMA.
        nc.scalar.dma_start(
            out=out[0 : P * n_blocks].rearrange("(p c) -> p c", p=P),
            in_=scores[:, 0:n_blocks],
        )
    else:
        # Transpose score columns so output DMAs are contiguous per row.
        for c0 in range(0, n_blocks, P):
            cc = min(P, n_blocks - c0)
            pt = psum_pool.tile([P, P], f32, tag="pt")
            nc.tensor.transpose(pt[0:cc, :], scores[:, c0 : c0 + cc], ident[:])
            sb = prod_pool.tile([P, P], f32, tag="scoresT")
            nc.vector.tensor_copy(sb[0:cc, :], pt[0:cc, :])
            nc.scalar.dma_start(
                out=out[c0 * P : (c0 + cc) * P].rearrange("(t p) -> t p", p=P),
                in_=sb[0:cc, :],
            )
    if tail:
        nc.scalar.dma_start(
            out=out[n_blocks * P : rows].rearrange("(p x) -> p x", x=1),
            in_=scores[0:tail, n_blocks : n_blocks + 1],
        )


def _get_program(rows, group):
    key = (rows, group)
    if key not in _CACHE:
        _CACHE[key] = _build(rows, group)
    return _CACHE[key]


def kernel(src_emb, dst_emb, M, rel_idx):
    from concourse.bass_utils import run_bass_kernel_spmd

    src_emb = np.asarray(src_emb, dtype=np.float32)
    dst_emb = np.asarray(dst_emb, dtype=np.float32)
    M = np.asarray(M, dtype=np.float32)
    rel = np.ascontiguousarray(M[int(rel_idx)]).reshape(1, D)

    nc = _get_program(ROWS, GROUP)
    in_maps = [
        {
            "src": np.ascontiguousarray(src_emb[i * ROWS : (i + 1) * ROWS]),
            "dst": np.ascontiguousarray(dst_emb[i * ROWS : (i + 1) * ROWS]),
            "rel": rel,
        }
        for i in range(N_CORES)
    ]
    res = run_bass_kernel_spmd(nc, in_maps, list(range(N_CORES)))
    out = np.concatenate([res.results[i]["out"] for i in range(N_CORES)])
    return out.reshape(B, 1, 1).astype(np.float32)
